# revision 38
# baseline (speedup 1.0000x reference)
"""Trainium2 Bass kernel for nn_Net_4715874091010 (2-layer NNConv GNN).

Strategy:
  - The edge MLPs (1->16->16->cin*cout, zero biases, edge_attr >= 0) are
    positively homogeneous: MLP(a) = a * MLP(1), so W_e = a_e * G with a
    fixed [cin, cout] matrix G per conv. Each conv collapses to
        y = segment_sum(a_e * P[src_e], dst) ,  P = x @ G1  (resp. relu(y1) @ G2)
    (a general per-edge-MLP fallback path is kept for safety).
  - Host preprocessing (index-only + tiny dense ops): relabel nodes by
    degree rank, sort edges by dst-rank, and lay messages out in a padded
    CSR format whose pad width is uniform across the 8 NeuronCores
    (groups of 8 node-tiles share one pad width) -> the same SPMD program
    serves all cores and padding inflation is ~7%.
  - Device (8 NeuronCores, SPMD, node-sharded): ONE unified 3-channel
    program serves both convs: stream the fp8-e4m3 message tensor from
    HBM, segmented tensor_reduce per node tile (f32 accumulate), then emit
    BOTH out_r = relu(y) (z, consumed after launch 1) and out_s =
    softmax(relu(y @ G2)) with the tiny G2 baked in as constants (the
    final output, consumed after launch 2 since segsum(a*z[src]) @ G2 ==
    (A@z) @ G2 by linearity). One walrus compile, one jitted executable
    called twice; the unused output of each launch is ignored. fp8 wire
    keeps rel-err ~3e-4 against the 2e-2 budget (f32 accumulation).
  - Wall-clock layout: a side thread runs the GIL-releasing waits (axon
    handshake, PJRT compile) while the main thread does the GIL-bound host
    work; the ISA cffi header parse warms in a third thread; message
    shards transfer per-core as each is scattered (the axon channel
    serializes transfers against compile RPCs, so launch 2's input hides
    fully under its own scatter).
  - Robustness: a device left wedged (NRT_EXEC_UNIT_UNRECOVERABLE) by an
    earlier tenant poisons the whole in-process PJRT client, so in-process
    retries never help. On any device failure the kernel re-runs itself in
    fresh subprocesses (fresh client each) with short backoff instead.
  - This toolchain cannot express a fast device-side gather (ext-ISA
    gpsimd ops fail codegen, indirect-DMA is slow per-row), so the
    index-driven gather/layout lives on the host; all streaming
    reduction and nonlinearities run on the NeuronCores. The launch path
    is the same axon/PJRT machinery bass_utils.run_bass_kernel_spmd uses
    (held persistently so the second conv skips retrace/recompile); the
    general fallback path calls run_bass_kernel_spmd directly.
"""
import os
import sys

sys.path.insert(0, "/opt/trn_rl_repo")

import numpy as np

N_NODES = 50000
F_IN = 16
H = 3
C = 4
N_CORES = 8
NT = 50176            # 392 tiles of 128 ranks
N_TILES = NT // 128   # 392
TPC = N_TILES // N_CORES  # 49 tile-groups (tiles per core)
CHUNK_COLS = 1536     # max per-channel columns per DMA chunk tile
MSG_DTYPE = "float8e4"  #

_IN_CHILD = os.environ.get("BASS_KERNEL_CHILD") == "1"

_tile_patched = False


def _patch_tile():
    """This walrus build rejects instructions with several sync waits
    ("Too many sync wait commands"); Tile's exit drain aggregates every
    outstanding sem wait onto one Drain. Split them across single-wait
    sync-engine NOPs (semantically identical)."""
    global _tile_patched
    if _tile_patched:
        return
    from concourse import mybir
    import concourse.tile as tile
    from concourse.vector_clock import ScopedClock

    def _drain_and_barrier(self, tick_clock, wait_clock):
        nc = self.nc
        # Waits execute on single-wait NOPs BEFORE the drain, so the drain
        # never runs while DMAs are still in flight.
        probe = nc.sync.nop(nofuse=True)
        wait_clock.add_sem_waits(
            probe.ins, ScopedClock({None: tick_clock.global_clock})
        )
        si = probe.ins.sync_info
        waits = list(si.on_wait or []) if si is not None else []
        if len(waits) > 1:
            upd = list(si.on_update or []) if si is not None else []
            probe.ins.sync_info = mybir.SyncInfo(on_wait=waits[:1], on_update=upd)
            for i in range(1, len(waits)):
                nop = nc.sync.nop(nofuse=True)
                nop.ins.sync_info = mybir.SyncInfo(on_wait=[waits[i]], on_update=[])
        nc.sync.drain()
        nc.all_engine_barrier()
        assert self.sems is not None
        popped = nc._tile_sem_poison_stack.pop()
        assert popped is self._sem_poison
        nc.clear_and_free_semaphores(list(self.sems.allocated().values()))
        nc.all_engine_barrier()

    tile.TileContext._drain_and_barrier = _drain_and_barrier
    _tile_patched = True


def _lrelu(x):
    return np.where(x > 0, x, np.float32(0.01) * x).astype(np.float32)


def _homogeneous_G(w1, w2, w3, cin, cout):
    v = _lrelu(w1)            # [1,16]
    u = _lrelu(v @ w2)        # [1,16]
    return (u @ w3).reshape(cin, cout).astype(np.float32)


class _Layout:
    """Degree-sorted node relabeling + SPMD-uniform padded CSR layout."""

    def __init__(self, dst):
        dst = dst.astype(np.int32, copy=False)
        deg = np.bincount(dst, minlength=NT).astype(np.int64)
        self.perm = np.argsort(deg, kind="stable")        # rank -> node id
        rank_of = np.empty(NT, np.int32)
        rank_of[self.perm] = np.arange(NT, dtype=np.int32)
        self.rank_of = rank_of
        rdst = rank_of[dst]
        # quicksort: order within equal dst ranks is arbitrary, which only
        # permutes summands within a node (k_s stays a valid 0..deg-1
        # labeling per node either way)
        self.order = np.argsort(rdst)                     # edge sort by dst rank
        rdst_s = rdst[self.order]
        deg_r = deg[self.perm]
        starts = np.zeros(NT + 1, np.int64)
        np.cumsum(deg_r, out=starts[1:])
        self.k_s = (np.arange(len(rdst_s), dtype=np.int64)
                    - starts[rdst_s]).astype(np.int32)
        t = rdst_s // 128
        self.i_core = (t % N_CORES).astype(np.int32)
        j = t // N_CORES
        self.p = (rdst_s % 128).astype(np.int32)
        tile_max = deg_r.reshape(N_TILES, 128).max(axis=1)
        Dg = tile_max.reshape(TPC, N_CORES).max(axis=1)
        Dg = np.maximum(4, ((Dg + 3) // 4) * 4).astype(np.int64)  # quantize
        self.Dg = Dg
        self.cum = np.zeros(TPC + 1, np.int64)
        np.cumsum(Dg, out=self.cum[1:])
        self.slots = int(self.cum[-1])
        self.Dj = Dg[j].astype(np.int32)
        self.j = j
        # global row (core-major) and per-channel-0 column of each edge slot
        self.row = self.i_core * 128 + self.p

    def build_M(self, vals_sorted, c_dim, dtype=np.float32):
        """vals_sorted: [E, c_dim] message values in dst-rank edge order.
        Returns [N_CORES*128, c_dim * slots] (channel-major per group);
        row r = core*128 + partition. Scatters in f32 (a cross-dtype fancy
        scatter into bf16 is ~2x slower than scatter + bulk astype)."""
        M = np.zeros((N_CORES * 128, c_dim * self.slots), np.float32)
        col0 = (c_dim * self.cum[self.j]).astype(np.int32) + self.k_s
        for cc in range(c_dim):
            M[self.row, col0 + cc * self.Dj] = vals_sorted[:, cc]
        return M if dtype == np.float32 else M.astype(dtype)

    def core_parts(self, c_dim):
        """Per-core edge partitions for the pipelined scatter->transfer
        path: (sel, p, col0, Dj) per core, cached. Cheap; computed in the
        compile-overlap window."""
        key = ("parts", c_dim)
        cached = getattr(self, "_parts_cache", None)
        if cached is not None and cached[0] == key:
            return cached[1]
        order = np.argsort(self.i_core, kind="stable")
        counts = np.bincount(self.i_core, minlength=N_CORES)
        bounds = np.zeros(N_CORES + 1, np.int64)
        np.cumsum(counts, out=bounds[1:])
        col0_all = (c_dim * self.cum[self.j]).astype(np.int32) + self.k_s
        parts = []
        for i in range(N_CORES):
            sel = order[bounds[i]:bounds[i + 1]]
            parts.append((sel, self.p[sel],
                          col0_all[sel], self.Dj[sel]))
        self._parts_cache = (key, parts)
        return parts

    def edge_views(self, a_s, src_s, rank_src):
        """Pre-permuted per-core (a, src, rank_src) edge arrays so each
        launch's message values are computed per-core without a full-size
        intermediate. Values are per-call (edge_attr may differ between
        calls); only the index permutations are cached (core_parts)."""
        return [(a_s[sel], src_s[sel], rank_src[sel])
                for sel, _, _, _ in self.core_parts(H)]

    def batches(self):
        """Runs of consecutive groups sharing one pad width."""
        out = []
        g = 0
        while g < TPC:
            D = int(self.Dg[g])
            ng = 1
            while g + ng < TPC and int(self.Dg[g + ng]) == D:
                ng += 1
            out.append((g, ng, D))
            g += ng
        return out

    def unrank_rows(self, arr_rank):
        """[NT, c] rank-order -> [N_NODES, c] node-id order."""
        out = np.empty((N_NODES, arr_rank.shape[1]), np.float32)
        valid = self.perm < N_NODES
        out[self.perm[valid]] = arr_rank[valid]
        return out


def _chunk_batches(batches, c_dim):
    """Pack (g0, ng, D) runs into DMA chunks of <= CHUNK_COLS*c cols."""
    chunks = []
    cur = []
    cur_cols = 0
    budget = CHUNK_COLS * c_dim
    for g0, ng, D in batches:
        while ng > 0:
            take = max(1, min(ng, (budget - cur_cols) // (c_dim * D)))
            if cur_cols > 0 and cur_cols + take * c_dim * D > budget:
                chunks.append(cur)
                cur, cur_cols = [], 0
                continue
            cur.append((g0, take, D))
            cur_cols += take * c_dim * D
            g0 += take
            ng -= take
            if cur_cols >= budget:
                chunks.append(cur)
                cur, cur_cols = [], 0
    if cur:
        chunks.append(cur)
    return chunks


# ---------------------------------------------------------------------------
# Unified fast-path program: H=3 message channels in (bf16), two outputs:
#   out_r = relu(segsum)                    [3ch]  (conv1's node features z)
#   out_s = softmax(relu(segsum @ G2))      [4ch]  (conv2's final output,
#           with the tiny G2 matrix baked in as immediate constants)
# The same program serves both launches: launch A feeds a*P1[src] and
# consumes out_r; launch B feeds a*z[src] and consumes out_s (segsum@G2 ==
# (A@z)@G2 by linearity). The unused output of each launch is ignored.
# ---------------------------------------------------------------------------


def _build_unified_program(layout, G2, reps=1):
    _patch_tile()
    from concourse import bass, mybir
    import concourse.tile as tile

    c_dim = H
    G2 = np.asarray(G2, np.float32).reshape(H, C)
    nc = bass.Bass("TRN2", target_bir_lowering=False, debug=False,
                   num_devices=N_CORES)
    nc.disable_frame_to_traceback = True
    F = c_dim * layout.slots
    msg_dt = getattr(mybir.dt, MSG_DTYPE)
    m_d = nc.dram_tensor("m", [128, F], msg_dt, kind="ExternalInput")
    outr_d = nc.dram_tensor("out_r", [128, TPC * H], mybir.dt.float32,
                            kind="ExternalOutput")
    outs_d = nc.dram_tensor("out_s", [128, TPC * C], mybir.dt.float32,
                            kind="ExternalOutput")
    chunks = _chunk_batches(layout.batches(), c_dim)
    cum = layout.cum

    with tile.TileContext(nc) as tc:
        with tc.tile_pool(name="mpool", bufs=3) as mpool, \
             tc.tile_pool(name="ypool", bufs=2) as ypool:
            for _ in range(reps):
                y = ypool.tile([128, TPC * H], mybir.dt.float32, tag="y")
                for chunk in chunks:
                    g_lo = chunk[0][0]
                    g_hi = chunk[-1][0] + chunk[-1][1]
                    c0 = c_dim * cum[g_lo]
                    ccols = int(c_dim * (cum[g_hi] - cum[g_lo]))
                    mt = mpool.tile([128, ccols], msg_dt, tag="m")
                    nc.sync.dma_start(out=mt[:], in_=m_d[:, c0 : c0 + ccols])
                    for g0, ng, D in chunk:
                        iv = mt[:, c_dim * (cum[g0] - cum[g_lo]):
                                   c_dim * (cum[g0] - cum[g_lo]) + ng * c_dim * D]
                        iv = iv.rearrange("p (n c k) -> p n c k", n=ng, c=c_dim, k=D)
                        ov = y[:, g0 * c_dim : (g0 + ng) * c_dim].rearrange(
                            "p (n c) -> p n c", n=ng, c=c_dim)
                        nc.vector.tensor_reduce(
                            out=ov, in_=iv, axis=mybir.AxisListType.X,
                            op=mybir.AluOpType.add)
                # out_r = relu(y)  (raw y preserved for the G2 stage)
                yr = ypool.tile([128, TPC * H], mybir.dt.float32, tag="yr")
                nc.vector.tensor_scalar_max(yr[:], y[:], 0.0)
                nc.sync.dma_start(out=outr_d[:], in_=yr[:])
                # t = y @ G2  (per 49-node group: [3] -> [4], G2 baked)
                t = ypool.tile([128, TPC * C], mybir.dt.float32, tag="t")
                tmp = ypool.tile([128, TPC], mybir.dt.float32, tag="tmp")
                ry = y[:].rearrange("p (n c) -> p n c", c=H)
                rt = t[:].rearrange("p (n c) -> p n c", c=C)
                for co in range(C):
                    nc.vector.tensor_scalar_mul(rt[:, :, co], ry[:, :, 0],
                                                float(G2[0, co]))
                    for ci in range(1, H):
                        nc.vector.tensor_scalar_mul(tmp[:], ry[:, :, ci],
                                                    float(G2[ci, co]))
                        nc.vector.tensor_tensor(out=rt[:, :, co],
                                                in0=rt[:, :, co], in1=tmp[:],
                                                op=mybir.AluOpType.add)
                nc.vector.tensor_scalar_max(t[:], t[:], 0.0)
                # class softmax over each 4-channel group of t
                e = ypool.tile([128, TPC * C], mybir.dt.float32, tag="e")
                nc.scalar.activation(out=e[:], in_=t[:],
                                     func=mybir.ActivationFunctionType.Exp)
                s = ypool.tile([128, TPC], mybir.dt.float32, tag="s")
                re = e[:].rearrange("p (n c) -> p n c", c=C)
                nc.vector.tensor_tensor(out=s[:], in0=re[:, :, 0],
                                        in1=re[:, :, 1], op=mybir.AluOpType.add)
                for cc in range(2, C):
                    nc.vector.tensor_tensor(out=s[:], in0=s[:],
                                            in1=re[:, :, cc],
                                            op=mybir.AluOpType.add)
                nc.vector.reciprocal(out=s[:], in_=s[:])
                o = ypool.tile([128, TPC * C], mybir.dt.float32, tag="o")
                ro = o[:].rearrange("p (n c) -> p n c", c=C)
                for cc in range(C):
                    nc.vector.tensor_tensor(out=ro[:, :, cc],
                                            in0=re[:, :, cc], in1=s[:],
                                            op=mybir.AluOpType.mult)
                nc.sync.dma_start(out=outs_d[:], in_=o[:])
    return nc


class _Runner:
    """Persistent jitted SPMD launcher for one Bass program — the same
    shard_map/bass_exec path run_bass_kernel_spmd takes under axon, but
    the compiled executable is kept so the second call skips retrace,
    walrus recompile and PJRT setup."""

    def __init__(self, nc):
        import jax
        from jax.sharding import Mesh, PartitionSpec
        from jax.experimental.shard_map import shard_map
        from concourse import bass2jax, mybir

        bass2jax.install_neuronx_cc_hook()
        self.jax = jax
        partition_name = (nc.partition_id_tensor.name
                          if nc.partition_id_tensor else None)
        in_names, out_names, out_avals, zero_shapes = [], [], [], []
        for alloc in nc.m.functions[0].allocations:
            if not isinstance(alloc, mybir.MemoryLocationSet):
                continue
            name = alloc.memorylocations[0].name
            if alloc.kind == "ExternalInput":
                if name != partition_name:
                    in_names.append(name)
            elif alloc.kind == "ExternalOutput":
                out_names.append(name)
                shape = tuple(alloc.tensor_shape)
                dtype = mybir.dt.np(alloc.dtype)
                out_avals.append(jax.core.ShapedArray(shape, dtype))
                zero_shapes.append((shape, dtype))
        self.in_names = in_names
        self.out_names = out_names
        self.zero_shapes = zero_shapes
        n_params = len(in_names)
        n_outs = len(out_names)
        in_names_all = in_names + out_names + (
            [partition_name] if partition_name else [])

        def _body(*args):
            operands = list(args)
            if partition_name is not None:
                operands.append(bass2jax.partition_id_tensor())
            return tuple(bass2jax._bass_exec_p.bind(
                *operands,
                out_avals=tuple(out_avals),
                in_names=tuple(in_names_all),
                out_names=tuple(out_names),
                lowering_input_output_aliases=(),
                sim_require_finite=True,
                sim_require_nnan=True,
                nc=nc,
            ))

        from jax.sharding import NamedSharding

        devices = jax.devices()[:N_CORES]
        mesh = Mesh(np.asarray(devices), ("core",))
        self.devices = devices
        self.sharding = NamedSharding(mesh, PartitionSpec("core"))
        in_specs = (PartitionSpec("core"),) * (n_params + n_outs)
        out_specs = (PartitionSpec("core"),) * n_outs
        fn = jax.jit(
            shard_map(_body, mesh=mesh, in_specs=in_specs,
                      out_specs=out_specs, check_rep=False),
            donate_argnums=tuple(range(n_params, n_params + n_outs)))
        # per-program input dtypes for the lower() signature
        self.in_structs = []
        for alloc in nc.m.functions[0].allocations:
            if (isinstance(alloc, mybir.MemoryLocationSet)
                    and alloc.kind == "ExternalInput"
                    and alloc.memorylocations[0].name in in_names):
                shape = tuple(alloc.tensor_shape)
                dtype = mybir.dt.np(alloc.dtype)
                self.in_structs.append(jax.ShapeDtypeStruct(
                    (N_CORES * shape[0],) + shape[1:], dtype))
        out_structs = [jax.ShapeDtypeStruct((N_CORES * s[0],) + tuple(s[1:]), d)
                       for s, d in zero_shapes]
        self.compiled = fn.lower(*self.in_structs, *out_structs).compile()
        # Donated output buffers transfer ~25ms per call when passed as
        # numpy; pre-put two single-use sets (one per launch) async now —
        # after the compile so they don't contend with the M1 transfers.
        self._zeros_pool = [self._make_zeros(), self._make_zeros()]

    def _make_zeros(self):
        return [self.jax.device_put(
                    np.zeros((N_CORES * s[0],) + tuple(s[1:]), d), self.sharding)
                for s, d in self.zero_shapes]

    def put_shards(self, shards):
        """shards: list of N_CORES [128, W] arrays already committed or
        numpy. Assembles the global sharded array without re-transfer."""
        jax = self.jax
        parts = [s if isinstance(s, jax.Array) else
                 jax.device_put(s, d) for s, d in zip(shards, self.devices)]
        gshape = (N_CORES * shards[0].shape[0],) + tuple(shards[0].shape[1:])
        return jax.make_array_from_single_device_arrays(
            gshape, self.sharding, parts)

    def run(self, glob_ins, fetch=None):
        """glob_ins: list of [N_CORES*128, ...] arrays (core-major rows,
        numpy or pre-device_put jax arrays). Returns dict name ->
        [N_CORES*128, cols] np.ndarray for names in `fetch` (default all)."""
        zeros = (self._zeros_pool.pop() if self._zeros_pool
                 else [np.zeros((N_CORES * s[0],) + tuple(s[1:]), d)
                       for s, d in self.zero_shapes])
        self._zeros_pool.insert(0, self._make_zeros())  # async refill
        outs = self.compiled(*glob_ins, *zeros)
        want = set(self.out_names if fetch is None else fetch)
        return {nm: np.asarray(o) for nm, o in zip(self.out_names, outs)
                if nm in want}


def _rank_to_node(arr_glob, c_dim):
    """[N_CORES*128, TPC*c] core-major device output -> [NT, c] rank order
    (rank = (j*8+i)*128+p)."""
    arr = arr_glob.reshape(N_CORES, 128, TPC, c_dim)
    return arr.transpose(2, 0, 1, 3).reshape(NT, c_dim)


def _scatter_put(layout, vals_fn, jaxmod, devices):
    """Per-core scatter of message values into the padded-CSR shard,
    issuing each shard's (async) device transfer as soon as it is built —
    the ~0.2s tunnel transfer hides under the remaining scatter work.
    vals_fn(core, sel) -> [len(sel), H] f32 values for that core's edges.
    Falls back to one numpy global array when devices aren't known yet."""
    import ml_dtypes

    W = H * layout.slots
    parts = layout.core_parts(H)
    shards = []
    for i in range(N_CORES):
        sel, p_i, col0_i, Dj_i = parts[i]
        v = vals_fn(i, sel)
        Mi = np.zeros((128, W), np.float32)
        for cc in range(H):
            Mi[p_i, col0_i + cc * Dj_i] = v[:, cc]
        Mi = Mi.astype(ml_dtypes.float8_e4m3 if MSG_DTYPE == "float8e4"
                       else ml_dtypes.bfloat16)
        if devices is not None:
            Mi = jaxmod.device_put(Mi, devices[i])
        shards.append(Mi)
    if devices is None:
        return np.concatenate(shards, axis=0)
    from jax.sharding import Mesh, NamedSharding, PartitionSpec

    sharding = NamedSharding(Mesh(np.asarray(devices), ("core",)),
                             PartitionSpec("core"))
    return jaxmod.make_array_from_single_device_arrays(
        (N_CORES * 128, W), sharding, shards)


_FAST_CACHE = {}


def _fast_kernel(x, a, src, dst, w):
    """Happy path: zero biases, nonneg edge_attr. Single unified program,
    launched twice. Raises on device trouble.

    Wall-clock layout (GIL-aware): the side thread holds only the
    GIL-releasing waits — the axon client handshake, then the PJRT/walrus
    compile — while the main thread runs the GIL-bound host work (edge
    sort, program build, message scatter) underneath them."""
    import hashlib
    import threading

    # jax first and alone: the side thread's devices() handshake is pure
    # RPC (GIL-free), so spawn it as early as possible and keep the
    # GIL-bound work (remaining imports, edge sort, program build, message
    # scatter) underneath it on the main thread.
    import jax

    G1 = _homogeneous_G(w["c1_w1"], w["c1_w2"], w["c1_w3"], F_IN, H)
    G2 = _homogeneous_G(w["c2_w1"], w["c2_w2"], w["c2_w3"], H, C)

    # G2 is baked into the program as constants, so it's part of the key
    key = hashlib.sha1(
        np.ascontiguousarray(dst).tobytes() + G2.tobytes()).hexdigest()
    cached = _FAST_CACHE.get(key)

    if cached is not None:
        layout, runner = cached
        a_s = a[layout.order, 0]
        src_s = src[layout.order]
        rank_src = layout.rank_of[src_s]
        views = layout.edge_views(a_s, src_s, rank_src)
        P1 = (x @ G1).astype(np.float32)
        M1 = _scatter_put(
            layout, lambda i, sel: views[i][0][:, None] * P1[views[i][1]],
            jax, runner.devices)
    else:
        state = {}
        program_ready = threading.Event()
        devices_ready = threading.Event()

        def _side():
            try:
                state["devices"] = jax.devices()[:N_CORES]  # axon handshake
                devices_ready.set()
                program_ready.wait()
                state["runner"] = _Runner(state["nc"])
            except Exception as e:     # surfaced at join time
                state["side_err"] = e
                devices_ready.set()

        ts = threading.Thread(target=_side, daemon=True)
        ts.start()

        # The first Bass() construction parses the ISA C headers via cffi
        # (~0.5s, functools.cache'd per process). Warm it in a thread: it
        # runs during the numpy layout work's GIL-free stretches.
        def _isa_warm():
            try:
                from concourse.isa import get_isa

                get_isa("TRN2")
            except Exception:
                pass   # non-fatal: Bass() will parse it itself

        ti = threading.Thread(target=_isa_warm, daemon=True)
        ti.start()

        from concourse import bass2jax  # noqa: F401

        layout = _Layout(dst)
        # numpy edge prep + M1 scatter BEFORE the program build: the axon
        # channel serializes transfers against the PJRT compile RPC, so
        # issue the shard transfers first — they drain during the program
        # build (GIL work, channel idle) instead of stalling launch A
        a_s = a[layout.order, 0]
        src_s = src[layout.order]
        rank_src = layout.rank_of[src_s]
        views = layout.edge_views(a_s, src_s, rank_src)
        P1 = (x @ G1).astype(np.float32)

        ti.join()
        state["nc"] = _build_unified_program(layout, G2)
        program_ready.set()

        devices = (state.get("devices")
                   if devices_ready.is_set() and "devices" in state else None)
        M1 = _scatter_put(
            layout, lambda i, sel: views[i][0][:, None] * P1[views[i][1]],
            jax, devices)

        ts.join()
        if "side_err" in state:
            raise state["side_err"]
        runner = state["runner"]
        _FAST_CACHE.clear()
        _FAST_CACHE[key] = (layout, runner)

    resA = runner.run([M1], fetch=["out_r"])
    z_rank = _rank_to_node(resA["out_r"], H)     # relu(y1) = z, rank order

    M2 = _scatter_put(
        layout, lambda i, sel: views[i][0][:, None] * z_rank[views[i][2]],
        jax, runner.devices)

    resB = runner.run([M2], fetch=["out_s"])
    out_rank = _rank_to_node(resB["out_s"], C)
    return layout.unrank_rows(out_rank)


def _subprocess_attempt(outp, code, env):
    """One fresh-process attempt. Returns the output array, or a
    (None, reason) tuple on failure."""
    import subprocess

    try:
        r = subprocess.run([sys.executable, "-c", code], env=env,
                           capture_output=True, timeout=150)
        if r.returncode == 0 and os.path.exists(outp):
            return np.load(outp)
        return None, (r.stderr or b"")[-1500:].decode("utf-8", "replace")
    except subprocess.TimeoutExpired:
        return None, "subprocess attempt timed out"
    except Exception as e:  # e.g. np.load of a partial file
        return None, f"{type(e).__name__}: {e}"


def _subprocess_retry(inputs, first_err, result_poll=None):
    """The in-process PJRT client is poisoned after a device error
    (NRT_EXEC_UNIT_UNRECOVERABLE etc.) — retry the whole kernel in fresh
    subprocesses (fresh client each) with short backoff until the device
    comes back. If `result_poll` is given, a still-running in-process
    attempt is polled between tries and wins if it finishes first."""
    import tempfile
    import time as _t

    kdir = os.path.dirname(os.path.abspath(__file__))
    tmpd = tempfile.mkdtemp(prefix="bassk_")
    inp = os.path.join(tmpd, "in.npz")
    outp = os.path.join(tmpd, "out.npy")
    np.savez(inp, **{k: np.asarray(v) for k, v in inputs.items()})
    code = (
        "import sys\n"
        f"sys.path.insert(0, {kdir!r})\n"
        "import numpy as np, kernel\n"
        f"z = np.load({inp!r})\n"
        "out = kernel.kernel(**{k: z[k] for k in z.files})\n"
        f"np.save({outp!r}, out)\n"
    )
    env = dict(os.environ, BASS_KERNEL_CHILD="1")
    deadline = _t.time() + 540.0
    delay = 3.0
    last = f"{type(first_err).__name__}: {first_err}" if first_err else "stalled"
    while True:
        if result_poll is not None and "out" in result_poll:
            return result_poll["out"]
        r = _subprocess_attempt(outp, code, env)
        if not isinstance(r, tuple):
            return r
        last = r[1] or last
        if _t.time() > deadline:
            raise RuntimeError(f"device retries exhausted; last error: {last}")
        _t.sleep(delay)
        delay = min(delay * 1.7, 20.0)


# ---------------------------------------------------------------------------
# General fallback (nonzero MLP biases / negative edge_attr): full per-edge
# MLP on host, two f32 programs via run_bass_kernel_spmd.
# ---------------------------------------------------------------------------


def _build_program(layout, c_dim, bias, softmax, reps=1):
    """Fallback device program: stream f32 M chunks, segmented reduce per
    tile-group, + bias + relu (+ softmax)."""
    _patch_tile()
    from concourse import bass, mybir
    import concourse.tile as tile

    nc = bass.Bass("TRN2", target_bir_lowering=False, debug=False,
                   num_devices=N_CORES)
    nc.disable_frame_to_traceback = True
    F = c_dim * layout.slots
    m_d = nc.dram_tensor("m", [128, F], mybir.dt.float32, kind="ExternalInput")
    out_cols = TPC * c_dim
    out_d = nc.dram_tensor("out", [128, out_cols], mybir.dt.float32,
                           kind="ExternalOutput")
    chunks = _chunk_batches(layout.batches(), c_dim)
    bias = np.asarray(bias, np.float32).reshape(c_dim)
    cum = layout.cum

    with tile.TileContext(nc) as tc:
        with tc.tile_pool(name="mpool", bufs=3) as mpool, \
             tc.tile_pool(name="ypool", bufs=2) as ypool:
            for _ in range(reps):
                y = ypool.tile([128, out_cols], mybir.dt.float32, tag="y")
                for chunk in chunks:
                    g_lo = chunk[0][0]
                    g_hi = chunk[-1][0] + chunk[-1][1]
                    c0 = c_dim * cum[g_lo]
                    ccols = int(c_dim * (cum[g_hi] - cum[g_lo]))
                    mt = mpool.tile([128, ccols], mybir.dt.float32, tag="m")
                    nc.sync.dma_start(out=mt[:], in_=m_d[:, c0 : c0 + ccols])
                    for g0, ng, D in chunk:
                        iv = mt[:, c_dim * (cum[g0] - cum[g_lo]):
                                   c_dim * (cum[g0] - cum[g_lo]) + ng * c_dim * D]
                        iv = iv.rearrange("p (n c k) -> p n c k", n=ng, c=c_dim, k=D)
                        ov = y[:, g0 * c_dim : (g0 + ng) * c_dim].rearrange(
                            "p (n c) -> p n c", n=ng, c=c_dim)
                        nc.vector.tensor_reduce(
                            out=ov, in_=iv, axis=mybir.AxisListType.X,
                            op=mybir.AluOpType.add)
                ry = y[:].rearrange("p (n c) -> p n c", c=c_dim)
                for cc in range(c_dim):
                    if float(bias[cc]) != 0.0:
                        nc.vector.tensor_scalar_add(ry[:, :, cc], ry[:, :, cc],
                                                    float(bias[cc]))
                nc.vector.tensor_scalar_max(y[:], y[:], 0.0)
                if softmax:
                    e = ypool.tile([128, out_cols], mybir.dt.float32, tag="e")
                    nc.scalar.activation(out=e[:], in_=y[:],
                                         func=mybir.ActivationFunctionType.Exp)
                    s = ypool.tile([128, TPC], mybir.dt.float32, tag="s")
                    re = e[:].rearrange("p (n c) -> p n c", c=c_dim)
                    nc.vector.tensor_tensor(out=s[:], in0=re[:, :, 0],
                                            in1=re[:, :, 1],
                                            op=mybir.AluOpType.add)
                    for cc in range(2, c_dim):
                        nc.vector.tensor_tensor(out=s[:], in0=s[:],
                                                in1=re[:, :, cc],
                                                op=mybir.AluOpType.add)
                    nc.vector.reciprocal(out=s[:], in_=s[:])
                    o = ypool.tile([128, out_cols], mybir.dt.float32, tag="o")
                    ro = o[:].rearrange("p (n c) -> p n c", c=c_dim)
                    for cc in range(c_dim):
                        nc.vector.tensor_tensor(out=ro[:, :, cc],
                                                in0=re[:, :, cc], in1=s[:],
                                                op=mybir.AluOpType.mult)
                    final = o
                else:
                    final = y
                nc.sync.dma_start(out=out_d[:], in_=final[:])
    return nc


def _run(nc, in_maps):
    from concourse.bass_utils import run_bass_kernel_spmd

    return run_bass_kernel_spmd(nc, in_maps, list(range(N_CORES)))


def _edge_msgs_general(P_nodes, a_col, src_sorted, w1, b1, w2, b2, w3, b3,
                       cin, cout):
    """Fallback: full per-edge MLP (handles nonzero biases / negative attr)."""
    h1 = _lrelu(a_col @ w1 + b1)
    h2 = _lrelu(h1 @ w2 + b2)
    W = (h2 @ w3 + b3).reshape(-1, cin, cout)
    return np.einsum("ei,eio->eo", P_nodes[src_sorted], W).astype(np.float32)


def _general_kernel(x, a, src, dst, w):
    layout = _Layout(dst)
    a_s_col = a[layout.order]
    src_s = src[layout.order]

    vals1 = _edge_msgs_general(x, a_s_col, src_s,
                               w["c1_w1"], w["c1_b1"], w["c1_w2"],
                               w["c1_b2"], w["c1_w3"], w["c1_b3"], F_IN, H)
    M1 = layout.build_M(vals1, H)
    ncA = _build_program(layout, H, w["c1_bias"], softmax=False)
    resA = _run(ncA, [{"m": M1[i * 128:(i + 1) * 128]} for i in range(N_CORES)])
    h_glob = np.concatenate([resA.results[i]["out"] for i in range(N_CORES)])
    h_rank = _rank_to_node(h_glob, H)
    h_node = np.zeros((NT, H), np.float32)
    h_node[layout.perm] = h_rank

    vals2 = _edge_msgs_general(h_node[:N_NODES], a_s_col, src_s,
                               w["c2_w1"], w["c2_b1"], w["c2_w2"],
                               w["c2_b2"], w["c2_w3"], w["c2_b3"], H, C)
    M2 = layout.build_M(vals2, C)
    ncB = _build_program(layout, C, w["c2_bias"], softmax=True)
    resB = _run(ncB, [{"m": M2[i * 128:(i + 1) * 128]} for i in range(N_CORES)])
    out_glob = np.concatenate([resB.results[i]["out"] for i in range(N_CORES)])
    return layout.unrank_rows(_rank_to_node(out_glob, C))


def kernel(**inputs):
    x = np.asarray(inputs["x"], np.float32)
    ei = np.asarray(inputs["edge_index"])
    src = ei[0].astype(np.int64)
    dst = ei[1].astype(np.int64)
    a = np.asarray(inputs["edge_attr"], np.float32)          # [E,1]

    w = {k: np.asarray(inputs[k], np.float32) for k in inputs
         if k.startswith(("c1_", "c2_"))}

    fast = (a.min() >= 0.0 and
            all(np.abs(w[k]).max() == 0.0
                for k in ("c1_b1", "c1_b2", "c1_b3", "c1_bias",
                          "c2_b1", "c2_b2", "c2_b3", "c2_bias")))

    impl = _fast_kernel if fast else _general_kernel
    if _IN_CHILD:
        return impl(x, a, src, dst, w)

    # Run the device path in a worker thread so a hung device RPC (as
    # opposed to one that errors out) can't pin kernel() forever: past a
    # deadline, fresh-subprocess attempts race the stuck worker and the
    # first finisher wins.
    import threading

    result = {}

    def _work():
        try:
            result["out"] = impl(x, a, src, dst, w)
        except Exception as e:
            result["err"] = e

    th = threading.Thread(target=_work, daemon=True)
    th.start()
    # Clean runs finish in ~3s; 90s means a wedged/stalled client, and a
    # fresh process often gets through while a stuck one keeps waiting.
    th.join(timeout=90.0)
    if "out" in result:
        return result["out"]
    return _subprocess_retry(inputs, result.get("err"), result_poll=result)


# revision 39
# speedup vs baseline: 1.4111x; 1.4111x over previous
"""Trainium2 Bass kernel for nn_Net_4715874091010 (2-layer NNConv GNN).

Strategy:
  - The edge MLPs (1->16->16->cin*cout, zero biases, edge_attr >= 0) are
    positively homogeneous: MLP(a) = a * MLP(1), so W_e = a_e * G with a
    fixed [cin, cout] matrix G per conv. Each conv collapses to
        y = segment_sum(a_e * P[src_e], dst) ,  P = x @ G1  (resp. relu(y1) @ G2)
    (a general per-edge-MLP fallback path is kept for safety).
  - Host preprocessing (index-only + tiny dense ops): relabel nodes by
    degree rank, sort edges by dst-rank, and lay messages out in a padded
    CSR format whose pad width is uniform across the 8 NeuronCores
    (groups of 8 node-tiles share one pad width) -> the same SPMD program
    serves all cores and padding inflation is ~7%.
  - Device (8 NeuronCores, SPMD, node-sharded): ONE unified 3-channel
    program serves both convs: stream the fp8-e4m3 message tensor from
    HBM, segmented tensor_reduce per node tile (f32 accumulate), then emit
    BOTH out_r = relu(y) (z, consumed after launch 1) and out_s =
    softmax(relu(y @ G2)) with the tiny G2 baked in as constants (the
    final output, consumed after launch 2 since segsum(a*z[src]) @ G2 ==
    (A@z) @ G2 by linearity). One walrus compile, one jitted executable
    called twice; the unused output of each launch is ignored. fp8 wire
    keeps rel-err ~3e-4 against the 2e-2 budget (f32 accumulation).
  - Wall-clock layout: a side thread runs the GIL-releasing waits (axon
    handshake, PJRT compile) while the main thread does the GIL-bound host
    work; the ISA cffi header parse warms in a third thread; message
    shards transfer per-core as each is scattered (the axon channel
    serializes transfers against compile RPCs, so launch 2's input hides
    fully under its own scatter).
  - Robustness: a device left wedged (NRT_EXEC_UNIT_UNRECOVERABLE) by an
    earlier tenant poisons the whole in-process PJRT client, so in-process
    retries never help. On any device failure the kernel re-runs itself in
    fresh subprocesses (fresh client each) with short backoff instead.
  - This toolchain cannot express a fast device-side gather (ext-ISA
    gpsimd ops fail codegen, indirect-DMA is slow per-row), so the
    index-driven gather/layout lives on the host; all streaming
    reduction and nonlinearities run on the NeuronCores. The launch path
    is the same axon/PJRT machinery bass_utils.run_bass_kernel_spmd uses
    (held persistently so the second conv skips retrace/recompile); the
    general fallback path calls run_bass_kernel_spmd directly.
"""
import os
import sys

sys.path.insert(0, "/opt/trn_rl_repo")

import numpy as np

N_NODES = 50000
F_IN = 16
H = 3
C = 4
N_CORES = 8
NT = 50176            # 392 tiles of 128 ranks
N_TILES = NT // 128   # 392
TPC = N_TILES // N_CORES  # 49 tile-groups (tiles per core)
CHUNK_COLS = 1536     # max per-channel columns per DMA chunk tile
MSG_DTYPE = "float8e4"  #

_IN_CHILD = os.environ.get("BASS_KERNEL_CHILD") == "1"

_tile_patched = False


def _patch_tile():
    """This walrus build rejects instructions with several sync waits
    ("Too many sync wait commands"); Tile's exit drain aggregates every
    outstanding sem wait onto one Drain. Split them across single-wait
    sync-engine NOPs (semantically identical)."""
    global _tile_patched
    if _tile_patched:
        return
    from concourse import mybir
    import concourse.tile as tile
    from concourse.vector_clock import ScopedClock

    def _drain_and_barrier(self, tick_clock, wait_clock):
        nc = self.nc
        # Waits execute on single-wait NOPs BEFORE the drain, so the drain
        # never runs while DMAs are still in flight.
        probe = nc.sync.nop(nofuse=True)
        wait_clock.add_sem_waits(
            probe.ins, ScopedClock({None: tick_clock.global_clock})
        )
        si = probe.ins.sync_info
        waits = list(si.on_wait or []) if si is not None else []
        if len(waits) > 1:
            upd = list(si.on_update or []) if si is not None else []
            probe.ins.sync_info = mybir.SyncInfo(on_wait=waits[:1], on_update=upd)
            for i in range(1, len(waits)):
                nop = nc.sync.nop(nofuse=True)
                nop.ins.sync_info = mybir.SyncInfo(on_wait=[waits[i]], on_update=[])
        nc.sync.drain()
        nc.all_engine_barrier()
        assert self.sems is not None
        popped = nc._tile_sem_poison_stack.pop()
        assert popped is self._sem_poison
        nc.clear_and_free_semaphores(list(self.sems.allocated().values()))
        nc.all_engine_barrier()

    tile.TileContext._drain_and_barrier = _drain_and_barrier
    _tile_patched = True


def _lrelu(x):
    return np.where(x > 0, x, np.float32(0.01) * x).astype(np.float32)


def _homogeneous_G(w1, w2, w3, cin, cout):
    v = _lrelu(w1)            # [1,16]
    u = _lrelu(v @ w2)        # [1,16]
    return (u @ w3).reshape(cin, cout).astype(np.float32)


class _Layout:
    """Degree-sorted node relabeling + SPMD-uniform padded CSR layout."""

    def __init__(self, dst):
        dst = dst.astype(np.int32, copy=False)
        deg = np.bincount(dst, minlength=NT).astype(np.int64)
        self.perm = np.argsort(deg, kind="stable")        # rank -> node id
        rank_of = np.empty(NT, np.int32)
        rank_of[self.perm] = np.arange(NT, dtype=np.int32)
        self.rank_of = rank_of
        rdst = rank_of[dst]
        # quicksort: order within equal dst ranks is arbitrary, which only
        # permutes summands within a node (k_s stays a valid 0..deg-1
        # labeling per node either way)
        self.order = np.argsort(rdst)                     # edge sort by dst rank
        rdst_s = rdst[self.order]
        deg_r = deg[self.perm]
        starts = np.zeros(NT + 1, np.int64)
        np.cumsum(deg_r, out=starts[1:])
        self.k_s = (np.arange(len(rdst_s), dtype=np.int64)
                    - starts[rdst_s]).astype(np.int32)
        t = rdst_s // 128
        self.i_core = (t % N_CORES).astype(np.int32)
        j = t // N_CORES
        self.p = (rdst_s % 128).astype(np.int32)
        tile_max = deg_r.reshape(N_TILES, 128).max(axis=1)
        Dg = tile_max.reshape(TPC, N_CORES).max(axis=1)
        Dg = np.maximum(4, ((Dg + 3) // 4) * 4).astype(np.int64)  # quantize
        self.Dg = Dg
        self.cum = np.zeros(TPC + 1, np.int64)
        np.cumsum(Dg, out=self.cum[1:])
        self.slots = int(self.cum[-1])
        self.Dj = Dg[j].astype(np.int32)
        self.j = j
        # global row (core-major) and per-channel-0 column of each edge slot
        self.row = self.i_core * 128 + self.p

    def build_M(self, vals_sorted, c_dim, dtype=np.float32):
        """vals_sorted: [E, c_dim] message values in dst-rank edge order.
        Returns [N_CORES*128, c_dim * slots] (channel-major per group);
        row r = core*128 + partition. Scatters in f32 (a cross-dtype fancy
        scatter into bf16 is ~2x slower than scatter + bulk astype)."""
        M = np.zeros((N_CORES * 128, c_dim * self.slots), np.float32)
        col0 = (c_dim * self.cum[self.j]).astype(np.int32) + self.k_s
        for cc in range(c_dim):
            M[self.row, col0 + cc * self.Dj] = vals_sorted[:, cc]
        return M if dtype == np.float32 else M.astype(dtype)

    def core_parts(self, c_dim):
        """Per-core edge partitions for the pipelined scatter->transfer
        path: (sel, p, col0, Dj) per core, cached. Cheap; computed in the
        compile-overlap window."""
        key = ("parts", c_dim)
        cached = getattr(self, "_parts_cache", None)
        if cached is not None and cached[0] == key:
            return cached[1]
        order = np.argsort(self.i_core, kind="stable")
        counts = np.bincount(self.i_core, minlength=N_CORES)
        bounds = np.zeros(N_CORES + 1, np.int64)
        np.cumsum(counts, out=bounds[1:])
        col0_all = (c_dim * self.cum[self.j]).astype(np.int32) + self.k_s
        parts = []
        for i in range(N_CORES):
            sel = order[bounds[i]:bounds[i + 1]]
            parts.append((sel, self.p[sel],
                          col0_all[sel], self.Dj[sel]))
        self._parts_cache = (key, parts)
        return parts

    def edge_views(self, a_s, src_s, rank_src):
        """Pre-permuted per-core (a, src, rank_src) edge arrays so each
        launch's message values are computed per-core without a full-size
        intermediate. Values are per-call (edge_attr may differ between
        calls); only the index permutations are cached (core_parts)."""
        return [(a_s[sel], src_s[sel], rank_src[sel])
                for sel, _, _, _ in self.core_parts(H)]

    def batches(self):
        """Runs of consecutive groups sharing one pad width."""
        out = []
        g = 0
        while g < TPC:
            D = int(self.Dg[g])
            ng = 1
            while g + ng < TPC and int(self.Dg[g + ng]) == D:
                ng += 1
            out.append((g, ng, D))
            g += ng
        return out

    def unrank_rows(self, arr_rank):
        """[NT, c] rank-order -> [N_NODES, c] node-id order."""
        out = np.empty((N_NODES, arr_rank.shape[1]), np.float32)
        valid = self.perm < N_NODES
        out[self.perm[valid]] = arr_rank[valid]
        return out


def _chunk_batches(batches, c_dim):
    """Pack (g0, ng, D) runs into DMA chunks of <= CHUNK_COLS*c cols."""
    chunks = []
    cur = []
    cur_cols = 0
    budget = CHUNK_COLS * c_dim
    for g0, ng, D in batches:
        while ng > 0:
            take = max(1, min(ng, (budget - cur_cols) // (c_dim * D)))
            if cur_cols > 0 and cur_cols + take * c_dim * D > budget:
                chunks.append(cur)
                cur, cur_cols = [], 0
                continue
            cur.append((g0, take, D))
            cur_cols += take * c_dim * D
            g0 += take
            ng -= take
            if cur_cols >= budget:
                chunks.append(cur)
                cur, cur_cols = [], 0
    if cur:
        chunks.append(cur)
    return chunks


# ---------------------------------------------------------------------------
# Unified fast-path program: H=3 message channels in (bf16), two outputs:
#   out_r = relu(segsum)                    [3ch]  (conv1's node features z)
#   out_s = softmax(relu(segsum @ G2))      [4ch]  (conv2's final output,
#           with the tiny G2 matrix baked in as immediate constants)
# The same program serves both launches: launch A feeds a*P1[src] and
# consumes out_r; launch B feeds a*z[src] and consumes out_s (segsum@G2 ==
# (A@z)@G2 by linearity). The unused output of each launch is ignored.
# ---------------------------------------------------------------------------


def _build_unified_program(layout, G2, reps=1):
    _patch_tile()
    from concourse import bass, mybir
    import concourse.tile as tile

    c_dim = H
    G2 = np.asarray(G2, np.float32).reshape(H, C)
    nc = bass.Bass("TRN2", target_bir_lowering=False, debug=False,
                   num_devices=N_CORES)
    nc.disable_frame_to_traceback = True
    F = c_dim * layout.slots
    msg_dt = getattr(mybir.dt, MSG_DTYPE)
    m_d = nc.dram_tensor("m", [128, F], msg_dt, kind="ExternalInput")
    outr_d = nc.dram_tensor("out_r", [128, TPC * H], mybir.dt.float32,
                            kind="ExternalOutput")
    outs_d = nc.dram_tensor("out_s", [128, TPC * C], mybir.dt.float32,
                            kind="ExternalOutput")
    chunks = _chunk_batches(layout.batches(), c_dim)
    cum = layout.cum

    with tile.TileContext(nc) as tc:
        with tc.tile_pool(name="mpool", bufs=3) as mpool, \
             tc.tile_pool(name="ypool", bufs=2) as ypool:
            for _ in range(reps):
                y = ypool.tile([128, TPC * H], mybir.dt.float32, tag="y")
                for chunk in chunks:
                    g_lo = chunk[0][0]
                    g_hi = chunk[-1][0] + chunk[-1][1]
                    c0 = c_dim * cum[g_lo]
                    ccols = int(c_dim * (cum[g_hi] - cum[g_lo]))
                    mt = mpool.tile([128, ccols], msg_dt, tag="m")
                    nc.sync.dma_start(out=mt[:], in_=m_d[:, c0 : c0 + ccols])
                    for g0, ng, D in chunk:
                        iv = mt[:, c_dim * (cum[g0] - cum[g_lo]):
                                   c_dim * (cum[g0] - cum[g_lo]) + ng * c_dim * D]
                        iv = iv.rearrange("p (n c k) -> p n c k", n=ng, c=c_dim, k=D)
                        ov = y[:, g0 * c_dim : (g0 + ng) * c_dim].rearrange(
                            "p (n c) -> p n c", n=ng, c=c_dim)
                        nc.vector.tensor_reduce(
                            out=ov, in_=iv, axis=mybir.AxisListType.X,
                            op=mybir.AluOpType.add)
                # out_r = relu(y)  (raw y preserved for the G2 stage)
                yr = ypool.tile([128, TPC * H], mybir.dt.float32, tag="yr")
                nc.vector.tensor_scalar_max(yr[:], y[:], 0.0)
                nc.sync.dma_start(out=outr_d[:], in_=yr[:])
                # t = y @ G2  (per 49-node group: [3] -> [4], G2 baked)
                t = ypool.tile([128, TPC * C], mybir.dt.float32, tag="t")
                tmp = ypool.tile([128, TPC], mybir.dt.float32, tag="tmp")
                ry = y[:].rearrange("p (n c) -> p n c", c=H)
                rt = t[:].rearrange("p (n c) -> p n c", c=C)
                for co in range(C):
                    nc.vector.tensor_scalar_mul(rt[:, :, co], ry[:, :, 0],
                                                float(G2[0, co]))
                    for ci in range(1, H):
                        nc.vector.tensor_scalar_mul(tmp[:], ry[:, :, ci],
                                                    float(G2[ci, co]))
                        nc.vector.tensor_tensor(out=rt[:, :, co],
                                                in0=rt[:, :, co], in1=tmp[:],
                                                op=mybir.AluOpType.add)
                nc.vector.tensor_scalar_max(t[:], t[:], 0.0)
                # class softmax over each 4-channel group of t
                e = ypool.tile([128, TPC * C], mybir.dt.float32, tag="e")
                nc.scalar.activation(out=e[:], in_=t[:],
                                     func=mybir.ActivationFunctionType.Exp)
                s = ypool.tile([128, TPC], mybir.dt.float32, tag="s")
                re = e[:].rearrange("p (n c) -> p n c", c=C)
                nc.vector.tensor_tensor(out=s[:], in0=re[:, :, 0],
                                        in1=re[:, :, 1], op=mybir.AluOpType.add)
                for cc in range(2, C):
                    nc.vector.tensor_tensor(out=s[:], in0=s[:],
                                            in1=re[:, :, cc],
                                            op=mybir.AluOpType.add)
                nc.vector.reciprocal(out=s[:], in_=s[:])
                o = ypool.tile([128, TPC * C], mybir.dt.float32, tag="o")
                ro = o[:].rearrange("p (n c) -> p n c", c=C)
                for cc in range(C):
                    nc.vector.tensor_tensor(out=ro[:, :, cc],
                                            in0=re[:, :, cc], in1=s[:],
                                            op=mybir.AluOpType.mult)
                nc.sync.dma_start(out=outs_d[:], in_=o[:])
    return nc


class _Runner:
    """Persistent jitted SPMD launcher for one Bass program — the same
    shard_map/bass_exec path run_bass_kernel_spmd takes under axon, but
    the compiled executable is kept so the second call skips retrace,
    walrus recompile and PJRT setup."""

    def __init__(self, nc):
        import jax
        from jax.sharding import Mesh, PartitionSpec
        from jax.experimental.shard_map import shard_map
        from concourse import bass2jax, mybir

        bass2jax.install_neuronx_cc_hook()
        self.jax = jax
        partition_name = (nc.partition_id_tensor.name
                          if nc.partition_id_tensor else None)
        in_names, out_names, out_avals, zero_shapes = [], [], [], []
        for alloc in nc.m.functions[0].allocations:
            if not isinstance(alloc, mybir.MemoryLocationSet):
                continue
            name = alloc.memorylocations[0].name
            if alloc.kind == "ExternalInput":
                if name != partition_name:
                    in_names.append(name)
            elif alloc.kind == "ExternalOutput":
                out_names.append(name)
                shape = tuple(alloc.tensor_shape)
                dtype = mybir.dt.np(alloc.dtype)
                out_avals.append(jax.core.ShapedArray(shape, dtype))
                zero_shapes.append((shape, dtype))
        self.in_names = in_names
        self.out_names = out_names
        self.zero_shapes = zero_shapes
        n_params = len(in_names)
        n_outs = len(out_names)
        in_names_all = in_names + out_names + (
            [partition_name] if partition_name else [])

        def _body(*args):
            operands = list(args)
            if partition_name is not None:
                operands.append(bass2jax.partition_id_tensor())
            return tuple(bass2jax._bass_exec_p.bind(
                *operands,
                out_avals=tuple(out_avals),
                in_names=tuple(in_names_all),
                out_names=tuple(out_names),
                lowering_input_output_aliases=(),
                sim_require_finite=True,
                sim_require_nnan=True,
                nc=nc,
            ))

        from jax.sharding import NamedSharding

        devices = jax.devices()[:N_CORES]
        mesh = Mesh(np.asarray(devices), ("core",))
        self.devices = devices
        self.sharding = NamedSharding(mesh, PartitionSpec("core"))
        in_specs = (PartitionSpec("core"),) * (n_params + n_outs)
        out_specs = (PartitionSpec("core"),) * n_outs
        fn = jax.jit(
            shard_map(_body, mesh=mesh, in_specs=in_specs,
                      out_specs=out_specs, check_rep=False),
            donate_argnums=tuple(range(n_params, n_params + n_outs)))
        # per-program input dtypes for the lower() signature
        self.in_structs = []
        for alloc in nc.m.functions[0].allocations:
            if (isinstance(alloc, mybir.MemoryLocationSet)
                    and alloc.kind == "ExternalInput"
                    and alloc.memorylocations[0].name in in_names):
                shape = tuple(alloc.tensor_shape)
                dtype = mybir.dt.np(alloc.dtype)
                self.in_structs.append(jax.ShapeDtypeStruct(
                    (N_CORES * shape[0],) + shape[1:], dtype))
        out_structs = [jax.ShapeDtypeStruct((N_CORES * s[0],) + tuple(s[1:]), d)
                       for s, d in zero_shapes]
        self.compiled = fn.lower(*self.in_structs, *out_structs).compile()
        # Donated output buffers transfer ~25ms per call when passed as
        # numpy; pre-put two single-use sets (one per launch) async now —
        # after the compile so they don't contend with the M1 transfers.
        self._zeros_pool = [self._make_zeros(), self._make_zeros()]

    def _make_zeros(self):
        return [self.jax.device_put(
                    np.zeros((N_CORES * s[0],) + tuple(s[1:]), d), self.sharding)
                for s, d in self.zero_shapes]

    def put_shards(self, shards):
        """shards: list of N_CORES [128, W] arrays already committed or
        numpy. Assembles the global sharded array without re-transfer."""
        jax = self.jax
        parts = [s if isinstance(s, jax.Array) else
                 jax.device_put(s, d) for s, d in zip(shards, self.devices)]
        gshape = (N_CORES * shards[0].shape[0],) + tuple(shards[0].shape[1:])
        return jax.make_array_from_single_device_arrays(
            gshape, self.sharding, parts)

    def run(self, glob_ins, fetch=None):
        """glob_ins: list of [N_CORES*128, ...] arrays (core-major rows,
        numpy or pre-device_put jax arrays). Returns dict name ->
        [N_CORES*128, cols] np.ndarray for names in `fetch` (default all)."""
        zeros = (self._zeros_pool.pop() if self._zeros_pool
                 else [np.zeros((N_CORES * s[0],) + tuple(s[1:]), d)
                       for s, d in self.zero_shapes])
        self._zeros_pool.insert(0, self._make_zeros())  # async refill
        outs = self.compiled(*glob_ins, *zeros)
        want = set(self.out_names if fetch is None else fetch)
        return {nm: np.asarray(o) for nm, o in zip(self.out_names, outs)
                if nm in want}


def _rank_to_node(arr_glob, c_dim):
    """[N_CORES*128, TPC*c] core-major device output -> [NT, c] rank order
    (rank = (j*8+i)*128+p)."""
    arr = arr_glob.reshape(N_CORES, 128, TPC, c_dim)
    return arr.transpose(2, 0, 1, 3).reshape(NT, c_dim)


def _scatter_put(layout, vals_fn, jaxmod, devices):
    """Per-core scatter of message values into the padded-CSR shard,
    issuing each shard's (async) device transfer as soon as it is built —
    the ~0.2s tunnel transfer hides under the remaining scatter work.
    vals_fn(core, sel) -> [len(sel), H] f32 values for that core's edges.
    Falls back to one numpy global array when devices aren't known yet."""
    import ml_dtypes

    W = H * layout.slots
    parts = layout.core_parts(H)
    shards = []
    for i in range(N_CORES):
        sel, p_i, col0_i, Dj_i = parts[i]
        v = vals_fn(i, sel)
        Mi = np.zeros((128, W), np.float32)
        for cc in range(H):
            Mi[p_i, col0_i + cc * Dj_i] = v[:, cc]
        Mi = Mi.astype(ml_dtypes.float8_e4m3 if MSG_DTYPE == "float8e4"
                       else ml_dtypes.bfloat16)
        if devices is not None:
            Mi = jaxmod.device_put(Mi, devices[i])
        shards.append(Mi)
    if devices is None:
        return np.concatenate(shards, axis=0)
    from jax.sharding import Mesh, NamedSharding, PartitionSpec

    sharding = NamedSharding(Mesh(np.asarray(devices), ("core",)),
                             PartitionSpec("core"))
    return jaxmod.make_array_from_single_device_arrays(
        (N_CORES * 128, W), sharding, shards)


_FAST_CACHE = {}


def _fast_kernel(x, a, src, dst, w):
    """Happy path: zero biases, nonneg edge_attr. Single unified program,
    launched twice. Raises on device trouble.

    Wall-clock layout (GIL-aware): the side thread holds only the
    GIL-releasing waits — the axon client handshake, then the PJRT/walrus
    compile — while the main thread runs the GIL-bound host work (edge
    sort, program build, message scatter) underneath them."""
    import hashlib
    import threading

    # jax first and alone: the side thread's devices() handshake is pure
    # RPC (GIL-free), so spawn it as early as possible and keep the
    # GIL-bound work (remaining imports, edge sort, program build, message
    # scatter) underneath it on the main thread.
    import jax

    G1 = _homogeneous_G(w["c1_w1"], w["c1_w2"], w["c1_w3"], F_IN, H)
    G2 = _homogeneous_G(w["c2_w1"], w["c2_w2"], w["c2_w3"], H, C)

    # G2 is baked into the program as constants, so it's part of the key
    key = hashlib.sha1(
        np.ascontiguousarray(dst).tobytes() + G2.tobytes()).hexdigest()
    cached = _FAST_CACHE.get(key)

    if cached is not None:
        layout, runner = cached
        a_s = a[layout.order, 0]
        src_s = src[layout.order]
        rank_src = layout.rank_of[src_s]
        views = layout.edge_views(a_s, src_s, rank_src)
        P1 = (x @ G1).astype(np.float32)
        M1 = _scatter_put(
            layout, lambda i, sel: views[i][0][:, None] * P1[views[i][1]],
            jax, runner.devices)
    else:
        state = {}
        program_ready = threading.Event()
        devices_ready = threading.Event()

        def _side():
            try:
                state["devices"] = jax.devices()[:N_CORES]  # axon handshake
                devices_ready.set()
                program_ready.wait()
                state["runner"] = _Runner(state["nc"])
            except Exception as e:     # surfaced at join time
                state["side_err"] = e
                devices_ready.set()

        ts = threading.Thread(target=_side, daemon=True)
        ts.start()

        # The first Bass() construction parses the ISA C headers via cffi
        # (~0.5s, functools.cache'd per process). Warm it in a thread: it
        # runs during the numpy layout work's GIL-free stretches.
        def _isa_warm():
            try:
                from concourse.isa import get_isa

                get_isa("TRN2")
                import concourse.tile  # noqa: F401  (program-build dep)
            except Exception:
                pass   # non-fatal: Bass() will parse it itself

        ti = threading.Thread(target=_isa_warm, daemon=True)
        ti.start()

        from concourse import bass2jax  # noqa: F401

        layout = _Layout(dst)
        # numpy edge prep + M1 scatter BEFORE the program build: the axon
        # channel serializes transfers against the PJRT compile RPC, so
        # issue the shard transfers first — they drain during the program
        # build (GIL work, channel idle) instead of stalling launch A
        a_s = a[layout.order, 0]
        src_s = src[layout.order]
        rank_src = layout.rank_of[src_s]
        views = layout.edge_views(a_s, src_s, rank_src)
        P1 = (x @ G1).astype(np.float32)

        ti.join()
        state["nc"] = _build_unified_program(layout, G2)
        program_ready.set()

        devices = (state.get("devices")
                   if devices_ready.is_set() and "devices" in state else None)
        M1 = _scatter_put(
            layout, lambda i, sel: views[i][0][:, None] * P1[views[i][1]],
            jax, devices)

        ts.join()
        if "side_err" in state:
            raise state["side_err"]
        runner = state["runner"]
        _FAST_CACHE.clear()
        _FAST_CACHE[key] = (layout, runner)

    resA = runner.run([M1], fetch=["out_r"])
    z_rank = _rank_to_node(resA["out_r"], H)     # relu(y1) = z, rank order

    M2 = _scatter_put(
        layout, lambda i, sel: views[i][0][:, None] * z_rank[views[i][2]],
        jax, runner.devices)

    resB = runner.run([M2], fetch=["out_s"])
    out_rank = _rank_to_node(resB["out_s"], C)
    return layout.unrank_rows(out_rank)


def _subprocess_attempt(outp, code, env):
    """One fresh-process attempt. Returns the output array, or a
    (None, reason) tuple on failure."""
    import subprocess

    try:
        r = subprocess.run([sys.executable, "-c", code], env=env,
                           capture_output=True, timeout=150)
        if r.returncode == 0 and os.path.exists(outp):
            return np.load(outp)
        return None, (r.stderr or b"")[-1500:].decode("utf-8", "replace")
    except subprocess.TimeoutExpired:
        return None, "subprocess attempt timed out"
    except Exception as e:  # e.g. np.load of a partial file
        return None, f"{type(e).__name__}: {e}"


def _subprocess_retry(inputs, first_err, result_poll=None):
    """The in-process PJRT client is poisoned after a device error
    (NRT_EXEC_UNIT_UNRECOVERABLE etc.) — retry the whole kernel in fresh
    subprocesses (fresh client each) with short backoff until the device
    comes back. If `result_poll` is given, a still-running in-process
    attempt is polled between tries and wins if it finishes first."""
    import tempfile
    import time as _t

    kdir = os.path.dirname(os.path.abspath(__file__))
    tmpd = tempfile.mkdtemp(prefix="bassk_")
    inp = os.path.join(tmpd, "in.npz")
    outp = os.path.join(tmpd, "out.npy")
    np.savez(inp, **{k: np.asarray(v) for k, v in inputs.items()})
    code = (
        "import sys\n"
        f"sys.path.insert(0, {kdir!r})\n"
        "import numpy as np, kernel\n"
        f"z = np.load({inp!r})\n"
        "out = kernel.kernel(**{k: z[k] for k in z.files})\n"
        f"np.save({outp!r}, out)\n"
    )
    env = dict(os.environ, BASS_KERNEL_CHILD="1")
    deadline = _t.time() + 540.0
    delay = 3.0
    last = f"{type(first_err).__name__}: {first_err}" if first_err else "stalled"
    while True:
        if result_poll is not None and "out" in result_poll:
            return result_poll["out"]
        r = _subprocess_attempt(outp, code, env)
        if not isinstance(r, tuple):
            return r
        last = r[1] or last
        if _t.time() > deadline:
            raise RuntimeError(f"device retries exhausted; last error: {last}")
        _t.sleep(delay)
        delay = min(delay * 1.7, 20.0)


# ---------------------------------------------------------------------------
# General fallback (nonzero MLP biases / negative edge_attr): full per-edge
# MLP on host, two f32 programs via run_bass_kernel_spmd.
# ---------------------------------------------------------------------------


def _build_program(layout, c_dim, bias, softmax, reps=1):
    """Fallback device program: stream f32 M chunks, segmented reduce per
    tile-group, + bias + relu (+ softmax)."""
    _patch_tile()
    from concourse import bass, mybir
    import concourse.tile as tile

    nc = bass.Bass("TRN2", target_bir_lowering=False, debug=False,
                   num_devices=N_CORES)
    nc.disable_frame_to_traceback = True
    F = c_dim * layout.slots
    m_d = nc.dram_tensor("m", [128, F], mybir.dt.float32, kind="ExternalInput")
    out_cols = TPC * c_dim
    out_d = nc.dram_tensor("out", [128, out_cols], mybir.dt.float32,
                           kind="ExternalOutput")
    chunks = _chunk_batches(layout.batches(), c_dim)
    bias = np.asarray(bias, np.float32).reshape(c_dim)
    cum = layout.cum

    with tile.TileContext(nc) as tc:
        with tc.tile_pool(name="mpool", bufs=3) as mpool, \
             tc.tile_pool(name="ypool", bufs=2) as ypool:
            for _ in range(reps):
                y = ypool.tile([128, out_cols], mybir.dt.float32, tag="y")
                for chunk in chunks:
                    g_lo = chunk[0][0]
                    g_hi = chunk[-1][0] + chunk[-1][1]
                    c0 = c_dim * cum[g_lo]
                    ccols = int(c_dim * (cum[g_hi] - cum[g_lo]))
                    mt = mpool.tile([128, ccols], mybir.dt.float32, tag="m")
                    nc.sync.dma_start(out=mt[:], in_=m_d[:, c0 : c0 + ccols])
                    for g0, ng, D in chunk:
                        iv = mt[:, c_dim * (cum[g0] - cum[g_lo]):
                                   c_dim * (cum[g0] - cum[g_lo]) + ng * c_dim * D]
                        iv = iv.rearrange("p (n c k) -> p n c k", n=ng, c=c_dim, k=D)
                        ov = y[:, g0 * c_dim : (g0 + ng) * c_dim].rearrange(
                            "p (n c) -> p n c", n=ng, c=c_dim)
                        nc.vector.tensor_reduce(
                            out=ov, in_=iv, axis=mybir.AxisListType.X,
                            op=mybir.AluOpType.add)
                ry = y[:].rearrange("p (n c) -> p n c", c=c_dim)
                for cc in range(c_dim):
                    if float(bias[cc]) != 0.0:
                        nc.vector.tensor_scalar_add(ry[:, :, cc], ry[:, :, cc],
                                                    float(bias[cc]))
                nc.vector.tensor_scalar_max(y[:], y[:], 0.0)
                if softmax:
                    e = ypool.tile([128, out_cols], mybir.dt.float32, tag="e")
                    nc.scalar.activation(out=e[:], in_=y[:],
                                         func=mybir.ActivationFunctionType.Exp)
                    s = ypool.tile([128, TPC], mybir.dt.float32, tag="s")
                    re = e[:].rearrange("p (n c) -> p n c", c=c_dim)
                    nc.vector.tensor_tensor(out=s[:], in0=re[:, :, 0],
                                            in1=re[:, :, 1],
                                            op=mybir.AluOpType.add)
                    for cc in range(2, c_dim):
                        nc.vector.tensor_tensor(out=s[:], in0=s[:],
                                                in1=re[:, :, cc],
                                                op=mybir.AluOpType.add)
                    nc.vector.reciprocal(out=s[:], in_=s[:])
                    o = ypool.tile([128, out_cols], mybir.dt.float32, tag="o")
                    ro = o[:].rearrange("p (n c) -> p n c", c=c_dim)
                    for cc in range(c_dim):
                        nc.vector.tensor_tensor(out=ro[:, :, cc],
                                                in0=re[:, :, cc], in1=s[:],
                                                op=mybir.AluOpType.mult)
                    final = o
                else:
                    final = y
                nc.sync.dma_start(out=out_d[:], in_=final[:])
    return nc


def _run(nc, in_maps):
    from concourse.bass_utils import run_bass_kernel_spmd

    return run_bass_kernel_spmd(nc, in_maps, list(range(N_CORES)))


def _edge_msgs_general(P_nodes, a_col, src_sorted, w1, b1, w2, b2, w3, b3,
                       cin, cout):
    """Fallback: full per-edge MLP (handles nonzero biases / negative attr)."""
    h1 = _lrelu(a_col @ w1 + b1)
    h2 = _lrelu(h1 @ w2 + b2)
    W = (h2 @ w3 + b3).reshape(-1, cin, cout)
    return np.einsum("ei,eio->eo", P_nodes[src_sorted], W).astype(np.float32)


def _general_kernel(x, a, src, dst, w):
    layout = _Layout(dst)
    a_s_col = a[layout.order]
    src_s = src[layout.order]

    vals1 = _edge_msgs_general(x, a_s_col, src_s,
                               w["c1_w1"], w["c1_b1"], w["c1_w2"],
                               w["c1_b2"], w["c1_w3"], w["c1_b3"], F_IN, H)
    M1 = layout.build_M(vals1, H)
    ncA = _build_program(layout, H, w["c1_bias"], softmax=False)
    resA = _run(ncA, [{"m": M1[i * 128:(i + 1) * 128]} for i in range(N_CORES)])
    h_glob = np.concatenate([resA.results[i]["out"] for i in range(N_CORES)])
    h_rank = _rank_to_node(h_glob, H)
    h_node = np.zeros((NT, H), np.float32)
    h_node[layout.perm] = h_rank

    vals2 = _edge_msgs_general(h_node[:N_NODES], a_s_col, src_s,
                               w["c2_w1"], w["c2_b1"], w["c2_w2"],
                               w["c2_b2"], w["c2_w3"], w["c2_b3"], H, C)
    M2 = layout.build_M(vals2, C)
    ncB = _build_program(layout, C, w["c2_bias"], softmax=True)
    resB = _run(ncB, [{"m": M2[i * 128:(i + 1) * 128]} for i in range(N_CORES)])
    out_glob = np.concatenate([resB.results[i]["out"] for i in range(N_CORES)])
    return layout.unrank_rows(_rank_to_node(out_glob, C))


def kernel(**inputs):
    x = np.asarray(inputs["x"], np.float32)
    ei = np.asarray(inputs["edge_index"])
    src = ei[0].astype(np.int64)
    dst = ei[1].astype(np.int64)
    a = np.asarray(inputs["edge_attr"], np.float32)          # [E,1]

    w = {k: np.asarray(inputs[k], np.float32) for k in inputs
         if k.startswith(("c1_", "c2_"))}

    fast = (a.min() >= 0.0 and
            all(np.abs(w[k]).max() == 0.0
                for k in ("c1_b1", "c1_b2", "c1_b3", "c1_bias",
                          "c2_b1", "c2_b2", "c2_b3", "c2_bias")))

    impl = _fast_kernel if fast else _general_kernel
    if _IN_CHILD:
        return impl(x, a, src, dst, w)

    # Run the device path in a worker thread so a hung device RPC (as
    # opposed to one that errors out) can't pin kernel() forever: past a
    # deadline, fresh-subprocess attempts race the stuck worker and the
    # first finisher wins.
    import threading

    result = {}

    def _work():
        try:
            result["out"] = impl(x, a, src, dst, w)
        except Exception as e:
            result["err"] = e

    th = threading.Thread(target=_work, daemon=True)
    th.start()
    # Clean runs finish in ~3s; 90s means a wedged/stalled client, and a
    # fresh process often gets through while a stuck one keeps waiting.
    th.join(timeout=90.0)
    if "out" in result:
        return result["out"]
    return _subprocess_retry(inputs, result.get("err"), result_poll=result)
